# revision 20
# baseline (speedup 1.0000x reference)
"""Multi-level dense 3D conv (AbstractConv3D) as a Trainium2 Bass kernel.

v5: dense 2-copy layout, dual-output-group, host realign, job groups.

Work is organized into GROUPS of 4 jobs; each job = (level, token-chunk)
and lives on one 32-row PE strip (rows 32c+16k+i = x[B0_j + u + k],
k=0,1). Big levels contribute 4 chunks to their own group; small levels
are merged (1-2 chunks each) so all four strips stay busy and the
16-tile (32c, 32j) diagonal rotation keeps the ~34ns matmul cadence.
A 512-token window takes 9 matmuls (32-deep, 32-wide stationary, one per
(dx,dy)): main output group = taps (dx,dy,-1),(dx,dy,0); aux group =
(dx,dy,+1) at token offset -2, realigned on the host. Input streams from
a host-prepacked fat [128, E] DRAM image per group at HBM line rate.
Outputs are dumped raw [128, OW]; the host adds main+aux+bias.
"""

import math
from contextlib import ExitStack

import numpy as np
import ml_dtypes

import concourse.bass as bass
import concourse.bacc as bacc
import concourse.mybir as mybir
import concourse.tile as tile
from concourse.bass_utils import run_bass_kernel_spmd

BF16 = ml_dtypes.bfloat16

RES = [16, 18, 20, 23, 26, 29, 32, 36, 40, 45, 50, 56, 63, 70, 76, 80]
L = 16
CIN = 16
COUT = 16
NCORES = 8
NWIN = 512

# Per-level geometry
S_L = [math.ceil(r / 8) for r in RES]  # x-slabs per core
P_L = [r + 1 for r in RES]  # padded y/z extent
T_L = [(s + 2) * p * p for s, p in zip(S_L, P_L)]  # piece tokens (w/ x-halo)
TC_L = [t - 2 * p * p for t, p in zip(T_L, P_L)]  # computed tokens
GUARD = 2048
GAP = 2048  # zero gap between levels
T_IN = GUARD + sum(T_L) + (L - 1) * GAP + GUARD
LVL_IN_BASE = [GUARD + sum(T_L[:i]) + i * GAP for i in range(L)]

_CACHE = {}


def _tap_index(dx, dy, dz):
    return (dx + 1) * 9 + (dy + 1) * 3 + (dz + 1)


def _mk_job(lvl, nch, c):
    P = P_L[lvl]
    qlen = math.ceil(TC_L[lvl] / nch)
    nwin = (qlen + 1) // NWIN + 1
    E = NWIN * nwin + 2 * P * P + 2 * P + 18
    B0 = LVL_IN_BASE[lvl] + c * qlen - P - 2
    assert B0 >= 0 and B0 + E + 2 <= T_IN, (lvl, c)
    return dict(lvl=lvl, c=c, qlen=qlen, nwin=nwin, E=E, B0=B0)


def _mk_groups():
    """Group spec: 4 jobs per group. Big levels -> 4 own chunks; small
    levels merged. Ordered big-first."""
    spec = [[(lvl, 4)] for lvl in (15, 14, 13, 12, 11, 10)]
    spec += [[(9, 2), (8, 2)], [(7, 2), (6, 2)], [(5, 2), (4, 2)],
             [(3, 1), (2, 1), (1, 1), (0, 1)]]
    groups = []
    fat = 0
    ow = 0
    for members in spec:
        jobs = []
        for lvl, nch in members:
            for c in range(nch):
                jobs.append(_mk_job(lvl, nch, c))
        assert len(jobs) == 4
        E = max(j["E"] for j in jobs)
        maxw = max(j["nwin"] for j in jobs)
        ngr = math.ceil(maxw / 4)
        groups.append(dict(jobs=jobs, E=E, ngr=ngr, fat=fat, dump=ow))
        fat += E
        ow += 2048 * ngr
    return groups, fat, ow


GROUPS, FAT_W, OW = _mk_groups()


def _build_program(levels=None):
    # `levels` kept for debug: restricts emitted groups to those touching
    # the given levels (host pack/unpack stays global).
    nc = bacc.Bacc("TRN2", target_bir_lowering=False, debug=False, num_devices=NCORES)
    x_ext = nc.declare_dram_parameter("x", [128, FAT_W], mybir.dt.bfloat16, isOutput=False)
    w_ext = nc.declare_dram_parameter("w", [128, L * 9 * 32], mybir.dt.bfloat16, isOutput=False)
    o_ext = nc.declare_dram_parameter("o", [128, OW], mybir.dt.bfloat16, isOutput=True)

    with tile.TileContext(nc) as tc, ExitStack() as ctx:
        w_pool = ctx.enter_context(tc.tile_pool(name="w", bufs=1))
        x_pool = ctx.enter_context(tc.tile_pool(name="x", bufs=3))
        ps_pool = ctx.enter_context(tc.tile_pool(name="ps", bufs=2, space="PSUM"))
        st_pool = ctx.enter_context(tc.tile_pool(name="st", bufs=3))

        w_sb = w_pool.tile([128, L * 9 * 32], mybir.dt.bfloat16)
        nc.sync.dma_start(w_sb[:], w_ext[:])

        for grp in GROUPS:
            if levels is not None and not any(
                j["lvl"] in levels for j in grp["jobs"]
            ):
                continue
            _emit_group(nc, x_pool, ps_pool, st_pool, w_sb, x_ext, o_ext, grp)
    nc.finalize()
    return nc


def _emit_group(nc, x_pool, ps_pool, st_pool, w_sb, x_ext, o_ext, grp):
    jobs, E = grp["jobs"], grp["E"]

    xt = x_pool.tile([128, E], mybir.dt.bfloat16, tag="xchunk")
    nseg = math.ceil(E / 28000)
    W = math.ceil(E / nseg)
    for s0 in range(0, E, W):
        sl = min(W, E - s0)
        nc.sync.dma_start(xt[:, s0 : s0 + sl], x_ext[:, grp["fat"] + s0 : grp["fat"] + s0 + sl])

    for g in range(grp["ngr"]):
        ps = ps_pool.tile([128, 2048], mybir.dt.float32, tag="ps")
        for m in range(9):
            dx, dy = divmod(m, 3)
            dx -= 1
            dy -= 1
            for d in range(4):
                for c in range(4):
                    job = jobs[c]
                    P = P_L[job["lvl"]]
                    PP = P * P
                    j = (c + d) % 4
                    w = 4 * g + j
                    if w >= job["nwin"]:
                        continue
                    nw = min(NWIN, job["qlen"] + 2 - NWIN * w)
                    off = PP + P + 2 + NWIN * w + dx * PP + dy * P - 1
                    slot = job["lvl"] * 9 + m
                    nc.tensor.matmul(
                        ps[32 * j : 32 * j + 32, 512 * c : 512 * c + nw],
                        w_sb[32 * c : 32 * c + 32, slot * 32 : slot * 32 + 32],
                        xt[32 * c : 32 * c + 32, off : off + nw],
                        start=(m == 0),
                        stop=(m == 8),
                        tile_position=(32 * c, 32 * j),
                    )
        st = st_pool.tile([128, 2048], mybir.dt.bfloat16, tag="stage")
        nc.scalar.copy(st[:, 0:1024], ps[:, 0:1024])
        nc.vector.tensor_copy(st[:, 1024:2048], ps[:, 1024:2048])
        gcol = grp["dump"] + 2048 * g
        nc.sync.dma_start(o_ext[:, gcol : gcol + 2048], st[:, 0:2048])


def _pack_inputs(input, weight):
    """Host-side pad/cast/transpose/shard. Returns per-core in_maps."""
    x = np.asarray(input)[0]  # [N, 16] f32
    wt = np.asarray(weight).reshape(L, 27, CIN, COUT)

    wb = np.zeros((128, L * 9 * 32), dtype=np.float32)
    for lvl in range(L):
        for m in range(9):
            dx, dy = divmod(m, 3)
            dx -= 1
            dy -= 1
            slot = lvl * 9 + m
            S = np.zeros((32, 32), dtype=np.float32)
            S[0:16, 0:16] = wt[lvl, _tap_index(dx, dy, -1)]
            S[16:32, 0:16] = wt[lvl, _tap_index(dx, dy, 0)]
            S[0:16, 16:32] = wt[lvl, _tap_index(dx, dy, 1)]
            for c in range(4):
                wb[32 * c : 32 * c + 32, slot * 32 : slot * 32 + 32] = S
    wb = wb.astype(BF16)

    xs = [np.zeros((16, T_IN), dtype=BF16) for _ in range(NCORES)]
    off = 0
    for lvl, r in enumerate(RES):
        P, s = P_L[lvl], S_L[lvl]
        g = x[off : off + r**3].reshape(r, r, r, CIN)
        off += r**3
        gp = np.zeros((CIN, 8 * s + 2, P, P), dtype=BF16)
        gp[:, 1 : r + 1, 0:r, 0:r] = g.transpose(3, 0, 1, 2)
        for i in range(NCORES):
            piece = gp[:, i * s : i * s + s + 2].reshape(CIN, T_L[lvl])
            xs[i][:, LVL_IN_BASE[lvl] : LVL_IN_BASE[lvl] + T_L[lvl]] = piece

    fats = []
    for i in range(NCORES):
        fat = np.zeros((128, FAT_W), dtype=BF16)
        for grp in GROUPS:
            fb, E = grp["fat"], grp["E"]
            for c, job in enumerate(grp["jobs"]):
                B0 = job["B0"]
                ee = min(E, T_IN - B0 - 1)
                for k in range(2):
                    fat[32 * c + 16 * k : 32 * c + 16 * k + 16, fb : fb + ee] = xs[i][
                        :, B0 + k : B0 + k + ee
                    ]
        fats.append(fat)

    return [{"x": fats[i], "w": wb} for i in range(NCORES)]


def _unpack_outputs(results, bias, levels=None):
    """Assemble [1, N, 16] f32 from per-core raw dumps (main+aux+bias)."""
    bs = np.asarray(bias, dtype=np.float32)
    n_total = sum(r**3 for r in RES)
    out = np.zeros((1, n_total, CIN), dtype=np.float32)
    lvl_out_off = np.concatenate([[0], np.cumsum([r**3 for r in RES])])
    # per level: list of (chunk_index_in_level -> (grp, c)) in chunk order
    lvl_jobs = {lvl: [] for lvl in range(L)}
    for grp in GROUPS:
        for c, job in enumerate(grp["jobs"]):
            lvl_jobs[job["lvl"]].append((grp, c, job))
    for lv in lvl_jobs.values():
        lv.sort(key=lambda t: t[2]["c"])

    for i in range(NCORES):
        o = np.asarray(results[i]["o"], dtype=np.float32)  # [128, OW]
        for lvl in range(L):
            if levels is not None and lvl not in levels:
                continue
            r = RES[lvl]
            P, s = P_L[lvl], S_L[lvl]
            TC = TC_L[lvl]
            n_i = min(s, r - i * s)
            if n_i <= 0:
                continue
            toks = np.zeros((16, TC), dtype=np.float32)
            for grp, c, job in lvl_jobs[lvl]:
                qlen, nwin = job["qlen"], job["nwin"]
                c0 = job["c"] * qlen
                if c0 >= TC:
                    continue
                ql = min(qlen, TC - c0)
                ngr = grp["ngr"]
                blk = o[:, grp["dump"] : grp["dump"] + 2048 * ngr].reshape(
                    128, ngr, 4, 512
                )
                mainf = np.zeros((16, nwin * 512), dtype=np.float32)
                auxf = np.zeros((16, nwin * 512 + 512), dtype=np.float32)
                for w in range(nwin):
                    g, j = divmod(w, 4)
                    mainf[:, 512 * w : 512 * w + 512] = blk[32 * j : 32 * j + 16, g, c]
                    auxf[:, 512 * w : 512 * w + 512] = blk[
                        32 * j + 16 : 32 * j + 32, g, c
                    ]
                toks[:, c0 : c0 + ql] = mainf[:, :ql] + auxf[:, 2 : 2 + ql]
            piece = toks.reshape(CIN, s, P, P)[:, 0:n_i, 0:r, 0:r]
            dst = lvl_out_off[lvl] + i * s * r * r
            out[0, dst : dst + n_i * r * r] = (
                piece.transpose(1, 2, 3, 0).reshape(-1, CIN) + bs[lvl]
            )
    return out


def run(input, offsets, resolutions, weight, bias, trace=False, levels=None, **trace_kw):
    key = ("nc", tuple(levels) if levels is not None else None)
    if key not in _CACHE:
        _CACHE[key] = _build_program(levels)
    nc = _CACHE[key]
    in_maps = _pack_inputs(input, weight)
    res = run_bass_kernel_spmd(nc, in_maps, list(range(NCORES)), trace=trace, **trace_kw)
    return _unpack_outputs(res.results, bias, levels), res


def kernel(input, offsets, resolutions, weight, bias):
    out, _ = run(input, offsets, resolutions, weight, bias)
    return out


# revision 21
# speedup vs baseline: 1.0758x; 1.0758x over previous
"""Multi-level dense 3D conv (AbstractConv3D) as a Trainium2 Bass kernel.

v5: dense 2-copy layout, dual-output-group, host realign, job groups.

Work is organized into GROUPS of 4 jobs; each job = (level, token-chunk)
and lives on one 32-row PE strip (rows 32c+16k+i = x[B0_j + u + k],
k=0,1). Big levels contribute 4 chunks to their own group; small levels
are merged (1-2 chunks each) so all four strips stay busy and the
16-tile (32c, 32j) diagonal rotation keeps the ~34ns matmul cadence.
A 512-token window takes 9 matmuls (32-deep, 32-wide stationary, one per
(dx,dy)): main output group = taps (dx,dy,-1),(dx,dy,0); aux group =
(dx,dy,+1) at token offset -2, realigned on the host. Input streams from
a host-prepacked fat [128, E] DRAM image per group at HBM line rate.
Outputs are dumped raw [128, OW]; the host adds main+aux+bias.
"""

import math
from contextlib import ExitStack

import numpy as np
import ml_dtypes

import concourse.bass as bass
import concourse.bacc as bacc
import concourse.mybir as mybir
import concourse.tile as tile
from concourse.bass_utils import run_bass_kernel_spmd

BF16 = ml_dtypes.bfloat16

RES = [16, 18, 20, 23, 26, 29, 32, 36, 40, 45, 50, 56, 63, 70, 76, 80]
L = 16
CIN = 16
COUT = 16
NCORES = 8
NWIN = 512

# Per-level geometry
S_L = [math.ceil(r / 8) for r in RES]  # x-slabs per core
P_L = [r + 1 for r in RES]  # padded y/z extent
T_L = [(s + 2) * p * p for s, p in zip(S_L, P_L)]  # piece tokens (w/ x-halo)
TC_L = [t - 2 * p * p for t, p in zip(T_L, P_L)]  # computed tokens
GUARD = 2048
GAP = 2048  # zero gap between levels
T_IN = GUARD + sum(T_L) + (L - 1) * GAP + GUARD
LVL_IN_BASE = [GUARD + sum(T_L[:i]) + i * GAP for i in range(L)]

_CACHE = {}


def _tap_index(dx, dy, dz):
    return (dx + 1) * 9 + (dy + 1) * 3 + (dz + 1)


def _mk_job(lvl, nch, c):
    P = P_L[lvl]
    qlen = math.ceil(TC_L[lvl] / nch)
    nwin = (qlen + 1) // NWIN + 1
    E = NWIN * nwin + 2 * P * P + 2 * P + 18
    B0 = LVL_IN_BASE[lvl] + c * qlen - P - 2
    assert B0 >= 0 and B0 + E + 2 <= T_IN, (lvl, c)
    return dict(lvl=lvl, c=c, qlen=qlen, nwin=nwin, E=E, B0=B0)


def _mk_groups():
    """Group spec: 4 jobs per group. Big levels -> 4 own chunks; small
    levels merged. Ordered big-first."""
    spec = [[(3, 1), (2, 1), (1, 1), (0, 1)], [(5, 2), (4, 2)],
            [(7, 2), (6, 2)], [(9, 2), (8, 2)]]
    spec += [[(lvl, 4)] for lvl in (10, 11, 12, 13, 14, 15)]
    groups = []
    fat = 0
    ow = 0
    for members in spec:
        jobs = []
        for lvl, nch in members:
            for c in range(nch):
                jobs.append(_mk_job(lvl, nch, c))
        assert len(jobs) == 4
        E = max(j["E"] for j in jobs)
        maxw = max(j["nwin"] for j in jobs)
        ngr = math.ceil(maxw / 4)
        groups.append(dict(jobs=jobs, E=E, ngr=ngr, fat=fat, dump=ow))
        fat += E
        ow += 2048 * ngr
    return groups, fat, ow


GROUPS, FAT_W, OW = _mk_groups()


def _build_program(levels=None):
    # `levels` kept for debug: restricts emitted groups to those touching
    # the given levels (host pack/unpack stays global).
    nc = bacc.Bacc("TRN2", target_bir_lowering=False, debug=False, num_devices=NCORES)
    x_ext = nc.declare_dram_parameter("x", [128, FAT_W], mybir.dt.bfloat16, isOutput=False)
    w_ext = nc.declare_dram_parameter("w", [128, L * 9 * 32], mybir.dt.bfloat16, isOutput=False)
    o_ext = nc.declare_dram_parameter("o", [128, OW], mybir.dt.bfloat16, isOutput=True)

    with tile.TileContext(nc) as tc, ExitStack() as ctx:
        w_pool = ctx.enter_context(tc.tile_pool(name="w", bufs=1))
        x_pool = ctx.enter_context(tc.tile_pool(name="x", bufs=3))
        ps_pool = ctx.enter_context(tc.tile_pool(name="ps", bufs=2, space="PSUM"))
        st_pool = ctx.enter_context(tc.tile_pool(name="st", bufs=3))

        w_sb = w_pool.tile([128, L * 9 * 32], mybir.dt.bfloat16)
        nc.sync.dma_start(w_sb[:], w_ext[:])

        for grp in GROUPS:
            if levels is not None and not any(
                j["lvl"] in levels for j in grp["jobs"]
            ):
                continue
            _emit_group(nc, x_pool, ps_pool, st_pool, w_sb, x_ext, o_ext, grp)
    nc.finalize()
    return nc


def _emit_group(nc, x_pool, ps_pool, st_pool, w_sb, x_ext, o_ext, grp):
    jobs, E = grp["jobs"], grp["E"]

    xt = x_pool.tile([128, E], mybir.dt.bfloat16, tag="xchunk")
    nseg = math.ceil(E / 28000)
    W = math.ceil(E / nseg)
    for s0 in range(0, E, W):
        sl = min(W, E - s0)
        nc.sync.dma_start(xt[:, s0 : s0 + sl], x_ext[:, grp["fat"] + s0 : grp["fat"] + s0 + sl])

    for g in range(grp["ngr"]):
        ps = ps_pool.tile([128, 2048], mybir.dt.float32, tag="ps")
        for m in range(9):
            dx, dy = divmod(m, 3)
            dx -= 1
            dy -= 1
            for d in range(4):
                for c in range(4):
                    job = jobs[c]
                    P = P_L[job["lvl"]]
                    PP = P * P
                    j = (c + d) % 4
                    w = 4 * g + j
                    if w >= job["nwin"]:
                        continue
                    nw = min(NWIN, job["qlen"] + 2 - NWIN * w)
                    off = PP + P + 2 + NWIN * w + dx * PP + dy * P - 1
                    slot = job["lvl"] * 9 + m
                    nc.tensor.matmul(
                        ps[32 * j : 32 * j + 32, 512 * c : 512 * c + nw],
                        w_sb[32 * c : 32 * c + 32, slot * 32 : slot * 32 + 32],
                        xt[32 * c : 32 * c + 32, off : off + nw],
                        start=(m == 0),
                        stop=(m == 8),
                        tile_position=(32 * c, 32 * j),
                    )
        st = st_pool.tile([128, 2048], mybir.dt.bfloat16, tag="stage")
        nc.scalar.copy(st[:, 0:1024], ps[:, 0:1024])
        nc.vector.tensor_copy(st[:, 1024:2048], ps[:, 1024:2048])
        gcol = grp["dump"] + 2048 * g
        nc.sync.dma_start(o_ext[:, gcol : gcol + 2048], st[:, 0:2048])


def _pack_inputs(input, weight):
    """Host-side pad/cast/transpose/shard. Returns per-core in_maps."""
    x = np.asarray(input)[0]  # [N, 16] f32
    wt = np.asarray(weight).reshape(L, 27, CIN, COUT)

    wb = np.zeros((128, L * 9 * 32), dtype=np.float32)
    for lvl in range(L):
        for m in range(9):
            dx, dy = divmod(m, 3)
            dx -= 1
            dy -= 1
            slot = lvl * 9 + m
            S = np.zeros((32, 32), dtype=np.float32)
            S[0:16, 0:16] = wt[lvl, _tap_index(dx, dy, -1)]
            S[16:32, 0:16] = wt[lvl, _tap_index(dx, dy, 0)]
            S[0:16, 16:32] = wt[lvl, _tap_index(dx, dy, 1)]
            for c in range(4):
                wb[32 * c : 32 * c + 32, slot * 32 : slot * 32 + 32] = S
    wb = wb.astype(BF16)

    xs = [np.zeros((16, T_IN), dtype=BF16) for _ in range(NCORES)]
    off = 0
    for lvl, r in enumerate(RES):
        P, s = P_L[lvl], S_L[lvl]
        g = x[off : off + r**3].reshape(r, r, r, CIN)
        off += r**3
        gp = np.zeros((CIN, 8 * s + 2, P, P), dtype=BF16)
        gp[:, 1 : r + 1, 0:r, 0:r] = g.transpose(3, 0, 1, 2)
        for i in range(NCORES):
            piece = gp[:, i * s : i * s + s + 2].reshape(CIN, T_L[lvl])
            xs[i][:, LVL_IN_BASE[lvl] : LVL_IN_BASE[lvl] + T_L[lvl]] = piece

    fats = []
    for i in range(NCORES):
        fat = np.zeros((128, FAT_W), dtype=BF16)
        for grp in GROUPS:
            fb, E = grp["fat"], grp["E"]
            for c, job in enumerate(grp["jobs"]):
                B0 = job["B0"]
                ee = min(E, T_IN - B0 - 1)
                for k in range(2):
                    fat[32 * c + 16 * k : 32 * c + 16 * k + 16, fb : fb + ee] = xs[i][
                        :, B0 + k : B0 + k + ee
                    ]
        fats.append(fat)

    return [{"x": fats[i], "w": wb} for i in range(NCORES)]


def _unpack_outputs(results, bias, levels=None):
    """Assemble [1, N, 16] f32 from per-core raw dumps (main+aux+bias)."""
    bs = np.asarray(bias, dtype=np.float32)
    n_total = sum(r**3 for r in RES)
    out = np.zeros((1, n_total, CIN), dtype=np.float32)
    lvl_out_off = np.concatenate([[0], np.cumsum([r**3 for r in RES])])
    # per level: list of (chunk_index_in_level -> (grp, c)) in chunk order
    lvl_jobs = {lvl: [] for lvl in range(L)}
    for grp in GROUPS:
        for c, job in enumerate(grp["jobs"]):
            lvl_jobs[job["lvl"]].append((grp, c, job))
    for lv in lvl_jobs.values():
        lv.sort(key=lambda t: t[2]["c"])

    for i in range(NCORES):
        o = np.asarray(results[i]["o"], dtype=np.float32)  # [128, OW]
        for lvl in range(L):
            if levels is not None and lvl not in levels:
                continue
            r = RES[lvl]
            P, s = P_L[lvl], S_L[lvl]
            TC = TC_L[lvl]
            n_i = min(s, r - i * s)
            if n_i <= 0:
                continue
            toks = np.zeros((16, TC), dtype=np.float32)
            for grp, c, job in lvl_jobs[lvl]:
                qlen, nwin = job["qlen"], job["nwin"]
                c0 = job["c"] * qlen
                if c0 >= TC:
                    continue
                ql = min(qlen, TC - c0)
                ngr = grp["ngr"]
                blk = o[:, grp["dump"] : grp["dump"] + 2048 * ngr].reshape(
                    128, ngr, 4, 512
                )
                mainf = np.zeros((16, nwin * 512), dtype=np.float32)
                auxf = np.zeros((16, nwin * 512 + 512), dtype=np.float32)
                for w in range(nwin):
                    g, j = divmod(w, 4)
                    mainf[:, 512 * w : 512 * w + 512] = blk[32 * j : 32 * j + 16, g, c]
                    auxf[:, 512 * w : 512 * w + 512] = blk[
                        32 * j + 16 : 32 * j + 32, g, c
                    ]
                toks[:, c0 : c0 + ql] = mainf[:, :ql] + auxf[:, 2 : 2 + ql]
            piece = toks.reshape(CIN, s, P, P)[:, 0:n_i, 0:r, 0:r]
            dst = lvl_out_off[lvl] + i * s * r * r
            out[0, dst : dst + n_i * r * r] = (
                piece.transpose(1, 2, 3, 0).reshape(-1, CIN) + bs[lvl]
            )
    return out


def run(input, offsets, resolutions, weight, bias, trace=False, levels=None, **trace_kw):
    key = ("nc", tuple(levels) if levels is not None else None)
    if key not in _CACHE:
        _CACHE[key] = _build_program(levels)
    nc = _CACHE[key]
    in_maps = _pack_inputs(input, weight)
    res = run_bass_kernel_spmd(nc, in_maps, list(range(NCORES)), trace=trace, **trace_kw)
    return _unpack_outputs(res.results, bias, levels), res


def kernel(input, offsets, resolutions, weight, bias):
    out, _ = run(input, offsets, resolutions, weight, bias)
    return out
